# revision 45
# baseline (speedup 1.0000x reference)
"""Trainium2 Bass kernel for nn_NodeAttentionPerMetaPath (GAT-style node attention).

Reference computation (N=8192, F_IN=256, d=64):
    h      = x @ trans                      # [N, d]
    e1     = h @ attn[:d];  e2 = h @ attn[d:]
    scores = leaky_relu(e1 + e2.T, 0.2)     # [N, N]
    masked = where(mask==0, -1e15, scores)
    out    = softmax(masked, axis=1) @ h    # [N, d]

Sharding: rows of the output across 8 cores (1024 rows each). Every core
receives the full xT (host-transposed, fp16) and computes h/e locally —
no collectives anywhere.

Algebraic restructuring (exact):
    exp(leaky(v)) = max(exp(v), exp(a*v))       (exp monotone, a<1)
    with v = e1[r] + e2[j], dropping per-row factors (softmax-invariant):
      P[r,j] ∝ mask * B2[j] * max(D[j], invC[r])
    where D = exp((1-a)e2), invC = exp(-(1-a)e1), B2 = exp(a*e2).

Layout tricks:
  * mask uploaded PRE-TRANSPOSED per core ([j, r]) and encoded as uint16
    0xFFFF/0x0000, so masking is a bitwise AND; bitcast to int32 halves
    the DVE element count (tensor_tensor runs 1x on trn2 DVE regardless
    of dtype — AND-on-int32 is the only way to get 2 f16/cycle).
  * per 128-j-chunk, D[j] and B2[j] are per-PARTITION scalars:
      v = max(invc_rep, D[j])      one 4x tensor_scalar
      p = v & mask                 one int32 AND (split DVE/GPSIMD)
      po[65, 1024] += haug[:,jc,:].T @ p    (haug = [B2*h | B2], so row
                                             64 of po is the denominator)
    No PE transposes, no PSUM->SBUF drains, no collectives.
"""

import os
from contextlib import ExitStack

import numpy as np

import concourse.bass as bass
import concourse.bacc as bacc
import concourse.mybir as mybir
import concourse.tile as tile
from concourse.bass_utils import run_bass_kernel_spmd
from concourse.masks import make_identity

f32 = mybir.dt.float32
f16 = mybir.dt.float16
u16 = mybir.dt.uint16
f8 = mybir.dt.float8e4
i32 = mybir.dt.int32

Exp = mybir.ActivationFunctionType.Exp
MULT = mybir.AluOpType.mult
MAX = mybir.AluOpType.max
AND = mybir.AluOpType.bitwise_and
ADD = mybir.AluOpType.add

N_CORES = 8
N = 8192
F_IN = 256
D = 64  # F_OUT
ALPHA = 0.2

R = N // N_CORES  # output rows per core
JC = N // 128  # j-chunks

# t = relu(invC-D) split: ScalarE ACT does r-columns [0:SA], DVE a
# dual-op tensor_scalar (sub, max 0) for [SA:R]
SA = 640
# q = m*t split per chunk: DVE multiplies r-columns [0:DS], GPSIMD [DS:R]
DS = 804

MASK_BUFS = 5  # in-flight mask DMA tiles of 4 j-chunks each


def build_kernel(ctx: ExitStack, tc: tile.TileContext, xT, xTo, mT, trans, a12, outT):
    nc = tc.nc

    singles = ctx.enter_context(tc.tile_pool(name="singles", bufs=1))
    maskp = ctx.enter_context(tc.tile_pool(name="maskp", bufs=MASK_BUFS))
    work = ctx.enter_context(tc.tile_pool(name="work", bufs=4))
    outp = ctx.enter_context(tc.tile_pool(name="outp", bufs=1))

    # ---- persistent tensors
    xT_sb = singles.tile([128, 2, N], f16)
    xTo_sb = singles.tile([128, 2, R], f16)
    # trans_w columns: [trans(0:64) | w1(64) | w2(65)] with w12 = trans @ a12
    trans_w = singles.tile([128, 2, D + 2], f16)
    a12_sb = singles.tile([D, 2], f16)
    ident = singles.tile([128, 128], f16)
    # haug columns: [B2*h(0:64) | B2(64)]; B2 carries a 1/16 so the f16
    # denominator row stays below 65504
    haug = singles.tile([128, JC, D + 1], f16)
    invc_rep = singles.tile([128, R], f16)
    # haugD = D[j] * haug  (the m*D term's stationary)
    haugD = singles.tile([128, JC, D + 1], f16)
    d_col = singles.tile([128, JC], f32)
    d2_col = singles.tile([128, JC], f32)
    neg_d_col = singles.tile([128, JC], f32)
    b2_col = singles.tile([128, JC], f32)
    ones1 = singles.tile([1, 128], f16)

    # xT first: it gates phase 1 (16 split dma_starts spread across queues)
    for k in range(16):
        sl = slice(k * 512, (k + 1) * 512)
        nc.sync.dma_start(
            out=xT_sb[:, :, sl], in_=xT.rearrange("(c p) j -> p c j", p=128)[:, :, sl]
        )
    nc.sync.dma_start(out=xTo_sb, in_=xTo.rearrange("(c p) r -> p c r", p=128))
    nc.sync.dma_start(
        out=trans_w[:, :, 0:D], in_=trans.rearrange("(c p) d -> p c d", p=128)
    )
    nc.sync.dma_start(out=a12_sb, in_=a12[:, :])

    # mask stream: 16 dma_starts of 4 j-chunks each -- small enough that
    # the consumer never stalls on a whole-tile DMA, big enough to amortize
    # descriptor-gen + semaphore propagation. Paced by tile-slot release.
    mask_tiles = []
    for g in range(JC // 4):
        mt = maskp.tile([128, 4, R], f16, tag="mt")
        nc.sync.dma_start(
            out=mt,
            in_=mT[g * 512 : (g + 1) * 512, :].rearrange("(c p) r -> p c r", p=128),
        )
        mask_tiles.append(mt)

    make_identity(nc, ident)
    nc.vector.memset(ones1, 1.0)

    # ---------------- phase 1 prologue: w12 and invC (own rows)
    with (
        tc.tile_pool(name="ph1", bufs=1) as ph1,
        tc.tile_pool(name="ph1ps", bufs=2, space="PSUM") as ph1ps,
    ):
        # w12 = trans @ a12 via PE: transpose trans chunks, then matmul
        transT = ph1.tile([D, 2, 128], f16)
        for fc in range(2):
            tp = ph1ps.tile([D, 128], f16, tag="tp", bufs=1)
            nc.tensor.transpose(tp, trans_w[:, fc, 0:D], ident)
            nc.vector.tensor_copy(transT[:, fc, :], tp)
        w12_ps = ph1ps.tile([128, 2, 2], f32, tag="w12", bufs=1)
        for fc in range(2):
            nc.tensor.matmul(
                w12_ps[:, fc, :], transT[:, fc, :], a12_sb, start=True, stop=True
            )
        nc.vector.tensor_copy(trans_w[:, :, D : D + 2], w12_ps)

        # invC for own rows (it gates every phase-2 op):
        # e1_own = w1.T @ xTo, exp(-0.8*x), broadcast via K=1 ones matmul
        invc_row = ph1.tile([1, R], f16)
        for h in range(2):
            hsl = slice(h * 512, (h + 1) * 512)
            e1o_ps = ph1ps.tile([1, 512], f32, tag="e1o", bufs=1)
            for fc in range(2):
                nc.tensor.matmul(
                    e1o_ps,
                    trans_w[:, fc, D : D + 1],
                    xTo_sb[:, fc, hsl],
                    start=(fc == 0),
                    stop=(fc == 1),
                )
            nc.scalar.activation(invc_row[:, hsl], e1o_ps, Exp, scale=-(1.0 - ALPHA))
        for h in range(2):
            hsl = slice(h * 512, (h + 1) * 512)
            ib_ps = ph1ps.tile([128, 512], f32, tag="ib", bufs=1)
            nc.tensor.matmul(ib_ps, ones1, invc_row[:, hsl], start=True, stop=True)
            nc.vector.tensor_copy(invc_rep[:, hsl], ib_ps)

    # ---------------- interleaved pipeline: he-groups (h/e/d/b2/haug/haugD
    # for 4 j-chunks) emitted LAG groups ahead of the phase-2 chunks that
    # consume them, so phase 1 and phase 2 overlap on every engine queue.
    heps = ctx.enter_context(tc.tile_pool(name="heps", bufs=2, space="PSUM"))
    ps_o = ctx.enter_context(tc.tile_pool(name="ps_o", bufs=1, space="PSUM"))
    po1 = ps_o.tile([D + 1, R], f32, tag="po1")
    po2 = ps_o.tile([D + 1, R], f32, tag="po2")
    Relu = mybir.ActivationFunctionType.Relu
    LAG = 4

    def emit_he_group(g):
        he_ps = heps.tile([128, 4, D + 2], f32, tag="he")
        for k in range(4):
            c = g * 4 + k
            csl = slice(c * 128, (c + 1) * 128)
            for fc in range(2):
                nc.tensor.matmul(
                    he_ps[:, k, :],
                    xT_sb[:, fc, csl],
                    trans_w[:, fc, :],
                    start=(fc == 0),
                    stop=(fc == 1),
                )
        csl4 = slice(g * 4, (g + 1) * 4)
        nc.scalar.activation(
            d_col[:, csl4], he_ps[:, :, D + 1], Exp, scale=1.0 - ALPHA
        )
        nc.scalar.activation(
            b2_col[:, csl4], he_ps[:, :, D + 1], Exp, scale=ALPHA
        )
        nc.vector.tensor_scalar(
            b2_col[:, csl4], b2_col[:, csl4], 1.0 / 16.0, None, MULT
        )
        nc.vector.tensor_scalar(
            neg_d_col[:, csl4], d_col[:, csl4], -1.0, None, MULT
        )
        nc.vector.tensor_tensor(
            d2_col[:, csl4], d_col[:, csl4], b2_col[:, csl4], MULT
        )
        nc.vector.tensor_copy(haug[:, csl4, D], b2_col[:, csl4])
        nc.scalar.copy(haugD[:, csl4, D], d2_col[:, csl4])
        for k in range(4):
            c = g * 4 + k
            if c % 2 == 0:
                nc.vector.tensor_scalar(
                    haug[:, c, 0:D],
                    he_ps[:, k, 0:D],
                    b2_col[:, c : c + 1],
                    None,
                    MULT,
                )
            else:
                nc.scalar.activation(
                    haug[:, c, 0:D],
                    he_ps[:, k, 0:D],
                    mybir.ActivationFunctionType.Copy,
                    scale=b2_col[:, c : c + 1],
                )
            nc.gpsimd.tensor_scalar(
                haugD[:, c, 0:D],
                haug[:, c, 0:D],
                d_col[:, c : c + 1],
                None,
                MULT,
            )

    def emit_chunk(jc):
        mt = mask_tiles[jc // 4][:, jc % 4, :]
        t = work.tile([128, R], f16, tag="t")
        q = work.tile([128, R], f16, tag="q")
        nc.scalar.activation(
            t[:, 0:SA], invc_rep[:, 0:SA], Relu, bias=neg_d_col[:, jc : jc + 1]
        )
        nc.vector.tensor_scalar(
            t[:, SA:],
            invc_rep[:, SA:],
            d_col[:, jc : jc + 1],
            0.0,
            mybir.AluOpType.subtract,
            MAX,
        )
        nc.vector.tensor_tensor(q[:, 0:DS], t[:, 0:DS], mt[:, 0:DS], MULT)
        nc.gpsimd.tensor_tensor(q[:, DS:], t[:, DS:], mt[:, DS:], MULT)
        for h in range(2):
            hsl = slice(h * 512, (h + 1) * 512)
            nc.tensor.matmul(
                po2[:, hsl],
                haug[:, jc, :],
                q[:, hsl],
                start=(jc == 0),
                stop=(jc == JC - 1),
            )
            nc.tensor.matmul(
                po1[:, hsl],
                haugD[:, jc, :],
                mt[:, hsl],
                start=(jc == 0),
                stop=(jc == JC - 1),
            )

    for g in range(LAG):
        emit_he_group(g)
    for g in range(JC // 4):
        if g + LAG < JC // 4:
            emit_he_group(g + LAG)
        for k in range(4):
            emit_chunk(g * 4 + k)

    # ---------------- normalize and store
    dn2 = outp.tile([1, R], f32)
    nc.scalar.copy(dn2, po2[D : D + 1, :])
    dn_row = outp.tile([1, R], f16)
    nc.vector.tensor_tensor(dn_row, po1[D : D + 1, :], dn2, ADD)
    osum = outp.tile([D, R], f32)
    nc.vector.tensor_copy(osum[:, 0:512], po1[0:D, 0:512])
    nc.scalar.copy(osum[:, 512:], po1[0:D, 512:])
    for h in range(2):
        hsl = slice(h * 512, (h + 1) * 512)
        nc.vector.tensor_tensor(osum[:, hsl], osum[:, hsl], po2[0:D, hsl], ADD)
    rr_sb = outp.tile([D, R], f32)
    with tc.tile_pool(name="ps_r", bufs=2, space="PSUM") as ps_r:
        for h in range(2):
            hsl = slice(h * 512, (h + 1) * 512)
            rr_ps = ps_r.tile([D, 512], f32, tag="rr")
            nc.tensor.matmul(
                rr_ps, ones1[:, 0:D], dn_row[:, hsl], start=True, stop=True
            )
            nc.vector.reciprocal_approx_fast(rr_sb[:, hsl], rr_ps)
    o_t = outp.tile([D, R], f32)
    nc.vector.tensor_tensor(o_t, osum, rr_sb, MULT)
    for k in range(4):
        sl = slice(k * 256, (k + 1) * 256)
        nc.sync.dma_start(out=outT[:, sl], in_=o_t[:, sl])


def build_nc():
    nc = bacc.Bacc("TRN2", num_devices=N_CORES)
    xT = nc.dram_tensor("xT", [F_IN, N], f16, kind="ExternalInput")
    xTo = nc.dram_tensor("xTo", [F_IN, R], f16, kind="ExternalInput")
    mT = nc.dram_tensor("mT", [N, R], f16, kind="ExternalInput")
    trans = nc.dram_tensor("trans", [F_IN, D], f16, kind="ExternalInput")
    a12 = nc.dram_tensor("a12", [D, 2], f16, kind="ExternalInput")
    outT = nc.dram_tensor("outT", [D, R], f32, kind="ExternalOutput")
    with ExitStack() as ctx:
        tc = ctx.enter_context(tile.TileContext(nc))
        build_kernel(
            ctx, tc, xT[:, :], xTo[:, :], mT[:, :], trans[:, :], a12[:, :], outT[:, :]
        )
    nc.compile()
    return nc


LAST_RESULTS = None


def kernel(x, mask, trans, attn, _trace=False):
    x = np.asarray(x)
    mask = np.asarray(mask)
    trans = np.asarray(trans)
    attn = np.asarray(attn)

    xT16 = np.ascontiguousarray(x.T.astype(np.float16))
    trans16 = np.ascontiguousarray(trans.astype(np.float16))
    a12_16 = np.ascontiguousarray(
        np.concatenate([attn[:D], attn[D:]], axis=1).astype(np.float16)
    )
    # mask as f16 1.0/0.0: feeds the PE directly (m*D term) and one
    # tensor_tensor (m*t term)
    mf = np.where(mask != 0, np.float16(1.0), np.float16(0.0))

    nc = build_nc()
    in_maps = []
    for c in range(N_CORES):
        rows = slice(c * R, (c + 1) * R)
        in_maps.append(
            {
                "xT": xT16,
                "xTo": np.ascontiguousarray(xT16[:, rows]),
                "mT": np.ascontiguousarray(mf[rows, :].T),
                "trans": trans16,
                "a12": a12_16,
            }
        )
    res = run_bass_kernel_spmd(nc, in_maps, list(range(N_CORES)), trace=_trace)
    global LAST_RESULTS
    LAST_RESULTS = res
    out = np.concatenate(
        [res.results[c]["outT"].T for c in range(N_CORES)], axis=0
    )
    return np.ascontiguousarray(out, dtype=np.float32)


if __name__ == "__main__":
    nc = build_nc()
    print("built OK")


# revision 46
# speedup vs baseline: 1.2728x; 1.2728x over previous
"""Trainium2 Bass kernel for nn_NodeAttentionPerMetaPath (GAT-style node attention).

Reference computation (N=8192, F_IN=256, d=64):
    h      = x @ trans                      # [N, d]
    e1     = h @ attn[:d];  e2 = h @ attn[d:]
    scores = leaky_relu(e1 + e2.T, 0.2)     # [N, N]
    masked = where(mask==0, -1e15, scores)
    out    = softmax(masked, axis=1) @ h    # [N, d]

Sharding: rows of the output across 8 cores (1024 rows each). Every core
receives the full xT (host-transposed, fp16) and computes h/e locally —
no collectives anywhere.

Algebraic restructuring (exact):
    exp(leaky(v)) = max(exp(v), exp(a*v))       (exp monotone, a<1)
    with v = e1[r] + e2[j], dropping per-row factors (softmax-invariant):
      P[r,j] ∝ mask * B2[j] * max(D[j], invC[r])
    where D = exp((1-a)e2), invC = exp(-(1-a)e1), B2 = exp(a*e2).

Layout tricks:
  * mask uploaded PRE-TRANSPOSED per core ([j, r]) and encoded as uint16
    0xFFFF/0x0000, so masking is a bitwise AND; bitcast to int32 halves
    the DVE element count (tensor_tensor runs 1x on trn2 DVE regardless
    of dtype — AND-on-int32 is the only way to get 2 f16/cycle).
  * per 128-j-chunk, D[j] and B2[j] are per-PARTITION scalars:
      v = max(invc_rep, D[j])      one 4x tensor_scalar
      p = v & mask                 one int32 AND (split DVE/GPSIMD)
      po[65, 1024] += haug[:,jc,:].T @ p    (haug = [B2*h | B2], so row
                                             64 of po is the denominator)
    No PE transposes, no PSUM->SBUF drains, no collectives.
"""

import os
from contextlib import ExitStack

import numpy as np

import concourse.bass as bass
import concourse.bacc as bacc
import concourse.mybir as mybir
import concourse.tile as tile
from concourse.bass_utils import run_bass_kernel_spmd
from concourse.masks import make_identity

f32 = mybir.dt.float32
f16 = mybir.dt.float16
u16 = mybir.dt.uint16
f8 = mybir.dt.float8e4
i32 = mybir.dt.int32

Exp = mybir.ActivationFunctionType.Exp
MULT = mybir.AluOpType.mult
MAX = mybir.AluOpType.max
AND = mybir.AluOpType.bitwise_and
ADD = mybir.AluOpType.add

N_CORES = 8
N = 8192
F_IN = 256
D = 64  # F_OUT
ALPHA = 0.2

R = N // N_CORES  # output rows per core
JC = N // 128  # j-chunks

# t = relu(invC-D) split: ScalarE ACT does r-columns [0:SA], DVE a
# dual-op tensor_scalar (sub, max 0) for [SA:R]
SA = 640
# q = m*t split per chunk: DVE multiplies r-columns [0:DS], GPSIMD [DS:R]
DS = 804

MASK_BUFS = 5  # in-flight mask DMA tiles of 4 j-chunks each


def build_kernel(ctx: ExitStack, tc: tile.TileContext, xT, xTo, mT, trans, a12, outT):
    nc = tc.nc

    singles = ctx.enter_context(tc.tile_pool(name="singles", bufs=1))
    maskp = ctx.enter_context(tc.tile_pool(name="maskp", bufs=MASK_BUFS))
    work = ctx.enter_context(tc.tile_pool(name="work", bufs=4))
    outp = ctx.enter_context(tc.tile_pool(name="outp", bufs=1))

    # ---- persistent tensors
    xT_sb = singles.tile([128, 2, N], f16)
    xTo_sb = singles.tile([128, 2, R], f16)
    # trans_w columns: [trans(0:64) | w1(64) | w2(65)] with w12 = trans @ a12
    trans_w = singles.tile([128, 2, D + 2], f16)
    a12_sb = singles.tile([D, 2], f16)
    ident = singles.tile([128, 128], f16)
    # haug columns: [B2*h(0:64) | B2(64)]; B2 carries a 1/16 so the f16
    # denominator row stays below 65504
    haug = singles.tile([128, JC, D + 1], f16)
    invc_rep = singles.tile([128, R], f16)
    # haugD = D[j] * haug  (the m*D term's stationary)
    haugD = singles.tile([128, JC, D + 1], f16)
    d_col = singles.tile([128, JC], f32)
    d2_col = singles.tile([128, JC], f32)
    neg_d_col = singles.tile([128, JC], f32)
    b2_col = singles.tile([128, JC], f32)
    ones1 = singles.tile([1, 128], f16)

    # xT first: it gates phase 1 (16 split dma_starts spread across queues)
    for k in range(16):
        sl = slice(k * 512, (k + 1) * 512)
        nc.sync.dma_start(
            out=xT_sb[:, :, sl], in_=xT.rearrange("(c p) j -> p c j", p=128)[:, :, sl]
        )
    nc.sync.dma_start(out=xTo_sb, in_=xTo.rearrange("(c p) r -> p c r", p=128))
    nc.sync.dma_start(
        out=trans_w[:, :, 0:D], in_=trans.rearrange("(c p) d -> p c d", p=128)
    )
    nc.sync.dma_start(out=a12_sb, in_=a12[:, :])

    # mask stream: 16 dma_starts of 4 j-chunks each -- small enough that
    # the consumer never stalls on a whole-tile DMA, big enough to amortize
    # descriptor-gen + semaphore propagation. Paced by tile-slot release.
    mask_tiles = []
    for g in range(JC // 4):
        mt = maskp.tile([128, 4, R], f16, tag="mt")
        nc.sync.dma_start(
            out=mt,
            in_=mT[g * 512 : (g + 1) * 512, :].rearrange("(c p) r -> p c r", p=128),
        )
        mask_tiles.append(mt)

    make_identity(nc, ident)
    nc.vector.memset(ones1, 1.0)

    # ---------------- phase 1 prologue: w12 and invC (own rows)
    with (
        tc.tile_pool(name="ph1", bufs=1) as ph1,
        tc.tile_pool(name="ph1ps", bufs=2, space="PSUM") as ph1ps,
    ):
        # w12 = trans @ a12 via PE: transpose trans chunks, then matmul
        transT = ph1.tile([D, 2, 128], f16)
        for fc in range(2):
            tp = ph1ps.tile([D, 128], f16, tag="tp", bufs=1)
            nc.tensor.transpose(tp, trans_w[:, fc, 0:D], ident)
            nc.vector.tensor_copy(transT[:, fc, :], tp)
        w12_ps = ph1ps.tile([128, 2, 2], f32, tag="w12", bufs=1)
        for fc in range(2):
            nc.tensor.matmul(
                w12_ps[:, fc, :], transT[:, fc, :], a12_sb, start=True, stop=True
            )
        nc.vector.tensor_copy(trans_w[:, :, D : D + 2], w12_ps)

        # invC for own rows (it gates every phase-2 op):
        # e1_own = w1.T @ xTo, exp(-0.8*x), broadcast via K=1 ones matmul
        invc_row = ph1.tile([1, R], f16)
        for h in range(2):
            hsl = slice(h * 512, (h + 1) * 512)
            e1o_ps = ph1ps.tile([1, 512], f32, tag="e1o", bufs=1)
            for fc in range(2):
                nc.tensor.matmul(
                    e1o_ps,
                    trans_w[:, fc, D : D + 1],
                    xTo_sb[:, fc, hsl],
                    start=(fc == 0),
                    stop=(fc == 1),
                )
            nc.scalar.activation(invc_row[:, hsl], e1o_ps, Exp, scale=-(1.0 - ALPHA))
        for h in range(2):
            hsl = slice(h * 512, (h + 1) * 512)
            ib_ps = ph1ps.tile([128, 512], f32, tag="ib", bufs=1)
            nc.tensor.matmul(ib_ps, ones1, invc_row[:, hsl], start=True, stop=True)
            nc.vector.tensor_copy(invc_rep[:, hsl], ib_ps)

    # ---------------- interleaved pipeline: he-groups (h/e/d/b2/haug/haugD
    # for 4 j-chunks) emitted LAG groups ahead of the phase-2 chunks that
    # consume them, so phase 1 and phase 2 overlap on every engine queue.
    heps = ctx.enter_context(tc.tile_pool(name="heps", bufs=2, space="PSUM"))
    ps_o = ctx.enter_context(tc.tile_pool(name="ps_o", bufs=1, space="PSUM"))
    po1 = ps_o.tile([D + 1, R], f32, tag="po1")
    po2 = ps_o.tile([D + 1, R], f32, tag="po2")
    Relu = mybir.ActivationFunctionType.Relu
    LAG = 4

    def emit_he_group(g):
        he_ps = heps.tile([128, 4, D + 2], f32, tag="he")
        for k in range(4):
            c = g * 4 + k
            csl = slice(c * 128, (c + 1) * 128)
            for fc in range(2):
                nc.tensor.matmul(
                    he_ps[:, k, :],
                    xT_sb[:, fc, csl],
                    trans_w[:, fc, :],
                    start=(fc == 0),
                    stop=(fc == 1),
                )
        csl4 = slice(g * 4, (g + 1) * 4)
        nc.scalar.activation(
            d_col[:, csl4], he_ps[:, :, D + 1], Exp, scale=1.0 - ALPHA
        )
        nc.scalar.activation(
            b2_col[:, csl4], he_ps[:, :, D + 1], Exp, scale=ALPHA
        )
        nc.vector.tensor_scalar(
            b2_col[:, csl4], b2_col[:, csl4], 1.0 / 16.0, None, MULT
        )
        nc.vector.tensor_scalar(
            neg_d_col[:, csl4], d_col[:, csl4], -1.0, None, MULT
        )
        nc.vector.tensor_tensor(
            d2_col[:, csl4], d_col[:, csl4], b2_col[:, csl4], MULT
        )
        nc.vector.tensor_copy(haug[:, csl4, D], b2_col[:, csl4])
        nc.scalar.copy(haugD[:, csl4, D], d2_col[:, csl4])
        for k in range(4):
            c = g * 4 + k
            if c % 2 == 0:
                nc.vector.tensor_scalar(
                    haug[:, c, 0:D],
                    he_ps[:, k, 0:D],
                    b2_col[:, c : c + 1],
                    None,
                    MULT,
                )
            else:
                nc.scalar.activation(
                    haug[:, c, 0:D],
                    he_ps[:, k, 0:D],
                    mybir.ActivationFunctionType.Copy,
                    scale=b2_col[:, c : c + 1],
                )
            if c % 2 == 0:
                nc.scalar.activation(
                    haugD[:, c, 0:D],
                    he_ps[:, k, 0:D],
                    mybir.ActivationFunctionType.Copy,
                    scale=d2_col[:, c : c + 1],
                )
            else:
                nc.vector.tensor_scalar(
                    haugD[:, c, 0:D],
                    he_ps[:, k, 0:D],
                    d2_col[:, c : c + 1],
                    None,
                    MULT,
                )

    def emit_chunk(jc):
        mt = mask_tiles[jc // 4][:, jc % 4, :]
        t = work.tile([128, R], f16, tag="t")
        q = work.tile([128, R], f16, tag="q")
        nc.scalar.activation(
            t[:, 0:SA], invc_rep[:, 0:SA], Relu, bias=neg_d_col[:, jc : jc + 1]
        )
        nc.vector.tensor_scalar(
            t[:, SA:],
            invc_rep[:, SA:],
            d_col[:, jc : jc + 1],
            0.0,
            mybir.AluOpType.subtract,
            MAX,
        )
        nc.vector.tensor_tensor(q[:, 0:DS], t[:, 0:DS], mt[:, 0:DS], MULT)
        nc.gpsimd.tensor_tensor(q[:, DS:], t[:, DS:], mt[:, DS:], MULT)
        for h in range(2):
            hsl = slice(h * 512, (h + 1) * 512)
            nc.tensor.matmul(
                po2[:, hsl],
                haug[:, jc, :],
                q[:, hsl],
                start=(jc == 0),
                stop=(jc == JC - 1),
            )
            nc.tensor.matmul(
                po1[:, hsl],
                haugD[:, jc, :],
                mt[:, hsl],
                start=(jc == 0),
                stop=(jc == JC - 1),
            )

    for g in range(LAG):
        emit_he_group(g)
    for g in range(JC // 4):
        if g + LAG < JC // 4:
            emit_he_group(g + LAG)
        for k in range(4):
            emit_chunk(g * 4 + k)

    # ---------------- normalize and store
    dn2 = outp.tile([1, R], f32)
    nc.scalar.copy(dn2, po2[D : D + 1, :])
    dn_row = outp.tile([1, R], f16)
    nc.vector.tensor_tensor(dn_row, po1[D : D + 1, :], dn2, ADD)
    osum = outp.tile([D, R], f32)
    nc.vector.tensor_copy(osum[:, 0:512], po1[0:D, 0:512])
    nc.scalar.copy(osum[:, 512:], po1[0:D, 512:])
    for h in range(2):
        hsl = slice(h * 512, (h + 1) * 512)
        nc.vector.tensor_tensor(osum[:, hsl], osum[:, hsl], po2[0:D, hsl], ADD)
    rr_sb = outp.tile([D, R], f32)
    with tc.tile_pool(name="ps_r", bufs=2, space="PSUM") as ps_r:
        for h in range(2):
            hsl = slice(h * 512, (h + 1) * 512)
            rr_ps = ps_r.tile([D, 512], f32, tag="rr")
            nc.tensor.matmul(
                rr_ps, ones1[:, 0:D], dn_row[:, hsl], start=True, stop=True
            )
            nc.vector.reciprocal_approx_fast(rr_sb[:, hsl], rr_ps)
    o_t = outp.tile([D, R], f32)
    nc.vector.tensor_tensor(o_t, osum, rr_sb, MULT)
    for k in range(4):
        sl = slice(k * 256, (k + 1) * 256)
        nc.sync.dma_start(out=outT[:, sl], in_=o_t[:, sl])


def build_nc():
    nc = bacc.Bacc("TRN2", num_devices=N_CORES)
    xT = nc.dram_tensor("xT", [F_IN, N], f16, kind="ExternalInput")
    xTo = nc.dram_tensor("xTo", [F_IN, R], f16, kind="ExternalInput")
    mT = nc.dram_tensor("mT", [N, R], f16, kind="ExternalInput")
    trans = nc.dram_tensor("trans", [F_IN, D], f16, kind="ExternalInput")
    a12 = nc.dram_tensor("a12", [D, 2], f16, kind="ExternalInput")
    outT = nc.dram_tensor("outT", [D, R], f32, kind="ExternalOutput")
    with ExitStack() as ctx:
        tc = ctx.enter_context(tile.TileContext(nc))
        build_kernel(
            ctx, tc, xT[:, :], xTo[:, :], mT[:, :], trans[:, :], a12[:, :], outT[:, :]
        )
    nc.compile()
    return nc


LAST_RESULTS = None


def kernel(x, mask, trans, attn, _trace=False):
    x = np.asarray(x)
    mask = np.asarray(mask)
    trans = np.asarray(trans)
    attn = np.asarray(attn)

    xT16 = np.ascontiguousarray(x.T.astype(np.float16))
    trans16 = np.ascontiguousarray(trans.astype(np.float16))
    a12_16 = np.ascontiguousarray(
        np.concatenate([attn[:D], attn[D:]], axis=1).astype(np.float16)
    )
    # mask as f16 1.0/0.0: feeds the PE directly (m*D term) and one
    # tensor_tensor (m*t term)
    mf = np.where(mask != 0, np.float16(1.0), np.float16(0.0))

    nc = build_nc()
    in_maps = []
    for c in range(N_CORES):
        rows = slice(c * R, (c + 1) * R)
        in_maps.append(
            {
                "xT": xT16,
                "xTo": np.ascontiguousarray(xT16[:, rows]),
                "mT": np.ascontiguousarray(mf[rows, :].T),
                "trans": trans16,
                "a12": a12_16,
            }
        )
    res = run_bass_kernel_spmd(nc, in_maps, list(range(N_CORES)), trace=_trace)
    global LAST_RESULTS
    LAST_RESULTS = res
    out = np.concatenate(
        [res.results[c]["outT"].T for c in range(N_CORES)], axis=0
    )
    return np.ascontiguousarray(out, dtype=np.float32)


if __name__ == "__main__":
    nc = build_nc()
    print("built OK")


# revision 47
# speedup vs baseline: 1.3236x; 1.0399x over previous
"""Trainium2 Bass kernel for nn_NodeAttentionPerMetaPath (GAT-style node attention).

Reference computation (N=8192, F_IN=256, d=64):
    h      = x @ trans                      # [N, d]
    e1     = h @ attn[:d];  e2 = h @ attn[d:]
    scores = leaky_relu(e1 + e2.T, 0.2)     # [N, N]
    masked = where(mask==0, -1e15, scores)
    out    = softmax(masked, axis=1) @ h    # [N, d]

Sharding: rows of the output across 8 cores (1024 rows each). Every core
receives the full xT (host-transposed, fp16) and computes h/e locally —
no collectives anywhere.

Algebraic restructuring (exact):
    exp(leaky(v)) = max(exp(v), exp(a*v))       (exp monotone, a<1)
    with v = e1[r] + e2[j], dropping per-row factors (softmax-invariant):
      P[r,j] ∝ mask * B2[j] * max(D[j], invC[r])
    where D = exp((1-a)e2), invC = exp(-(1-a)e1), B2 = exp(a*e2).

Layout tricks:
  * mask uploaded PRE-TRANSPOSED per core ([j, r]) and encoded as uint16
    0xFFFF/0x0000, so masking is a bitwise AND; bitcast to int32 halves
    the DVE element count (tensor_tensor runs 1x on trn2 DVE regardless
    of dtype — AND-on-int32 is the only way to get 2 f16/cycle).
  * per 128-j-chunk, D[j] and B2[j] are per-PARTITION scalars:
      v = max(invc_rep, D[j])      one 4x tensor_scalar
      p = v & mask                 one int32 AND (split DVE/GPSIMD)
      po[65, 1024] += haug[:,jc,:].T @ p    (haug = [B2*h | B2], so row
                                             64 of po is the denominator)
    No PE transposes, no PSUM->SBUF drains, no collectives.
"""

import os
from contextlib import ExitStack

import numpy as np

import concourse.bass as bass
import concourse.bacc as bacc
import concourse.mybir as mybir
import concourse.tile as tile
from concourse.bass_utils import run_bass_kernel_spmd
from concourse.masks import make_identity

f32 = mybir.dt.float32
f16 = mybir.dt.float16
u16 = mybir.dt.uint16
f8 = mybir.dt.float8e4
i32 = mybir.dt.int32

Exp = mybir.ActivationFunctionType.Exp
MULT = mybir.AluOpType.mult
MAX = mybir.AluOpType.max
AND = mybir.AluOpType.bitwise_and
ADD = mybir.AluOpType.add

N_CORES = 8
N = 8192
F_IN = 256
D = 64  # F_OUT
ALPHA = 0.2

R = N // N_CORES  # output rows per core
JC = N // 128  # j-chunks

# hybrid column split at CS=512: columns [0:CS] use the two-pass form
# (t = relu(invC-D) on ScalarE; po1 adds the m*D term via matmul), columns
# [CS:R] the one-pass form (t = max(invC, D) on DVE; po1 not needed).
CS = 512
# q = m*t split per chunk: DVE multiplies r-columns [0:DS], GPSIMD [DS:R]
DS = 788

MASK_BUFS = 5  # in-flight mask DMA tiles of 4 j-chunks each


def build_kernel(ctx: ExitStack, tc: tile.TileContext, xT, xTo, mT, trans, a12, outT):
    nc = tc.nc

    singles = ctx.enter_context(tc.tile_pool(name="singles", bufs=1))
    maskp = ctx.enter_context(tc.tile_pool(name="maskp", bufs=MASK_BUFS))
    work = ctx.enter_context(tc.tile_pool(name="work", bufs=4))
    outp = ctx.enter_context(tc.tile_pool(name="outp", bufs=1))

    # ---- persistent tensors
    xT_sb = singles.tile([128, 2, N], f16)
    xTo_sb = singles.tile([128, 2, R], f16)
    # trans_w columns: [trans(0:64) | w1(64) | w2(65)] with w12 = trans @ a12
    trans_w = singles.tile([128, 2, D + 2], f16)
    a12_sb = singles.tile([D, 2], f16)
    ident = singles.tile([128, 128], f16)
    # haug columns: [B2*h(0:64) | B2(64)]; B2 carries a 1/16 so the f16
    # denominator row stays below 65504
    haug = singles.tile([128, JC, D + 1], f16)
    invc_rep = singles.tile([128, R], f16)
    # haugD = D[j] * haug  (the m*D term's stationary)
    haugD = singles.tile([128, JC, D + 1], f16)
    d_col = singles.tile([128, JC], f32)
    d2_col = singles.tile([128, JC], f32)
    neg_d_col = singles.tile([128, JC], f32)
    b2_col = singles.tile([128, JC], f32)
    ones1 = singles.tile([1, 128], f16)

    # xT first: it gates phase 1 (16 split dma_starts spread across queues)
    for k in range(16):
        sl = slice(k * 512, (k + 1) * 512)
        nc.sync.dma_start(
            out=xT_sb[:, :, sl], in_=xT.rearrange("(c p) j -> p c j", p=128)[:, :, sl]
        )
    nc.sync.dma_start(out=xTo_sb, in_=xTo.rearrange("(c p) r -> p c r", p=128))
    nc.sync.dma_start(
        out=trans_w[:, :, 0:D], in_=trans.rearrange("(c p) d -> p c d", p=128)
    )
    nc.sync.dma_start(out=a12_sb, in_=a12[:, :])

    # mask stream: 16 dma_starts of 4 j-chunks each -- small enough that
    # the consumer never stalls on a whole-tile DMA, big enough to amortize
    # descriptor-gen + semaphore propagation. Paced by tile-slot release.
    mask_tiles = []
    for g in range(JC // 4):
        mt = maskp.tile([128, 4, R], f16, tag="mt")
        nc.sync.dma_start(
            out=mt,
            in_=mT[g * 512 : (g + 1) * 512, :].rearrange("(c p) r -> p c r", p=128),
        )
        mask_tiles.append(mt)

    make_identity(nc, ident)
    nc.vector.memset(ones1, 1.0)

    # ---------------- phase 1 prologue: w12 and invC (own rows)
    with (
        tc.tile_pool(name="ph1", bufs=1) as ph1,
        tc.tile_pool(name="ph1ps", bufs=2, space="PSUM") as ph1ps,
    ):
        # w12 = trans @ a12 via PE: transpose trans chunks, then matmul
        transT = ph1.tile([D, 2, 128], f16)
        for fc in range(2):
            tp = ph1ps.tile([D, 128], f16, tag="tp", bufs=1)
            nc.tensor.transpose(tp, trans_w[:, fc, 0:D], ident)
            nc.vector.tensor_copy(transT[:, fc, :], tp)
        w12_ps = ph1ps.tile([128, 2, 2], f32, tag="w12", bufs=1)
        for fc in range(2):
            nc.tensor.matmul(
                w12_ps[:, fc, :], transT[:, fc, :], a12_sb, start=True, stop=True
            )
        nc.vector.tensor_copy(trans_w[:, :, D : D + 2], w12_ps)

        # invC for own rows (it gates every phase-2 op):
        # e1_own = w1.T @ xTo, exp(-0.8*x), broadcast via K=1 ones matmul
        invc_row = ph1.tile([1, R], f16)
        for h in range(2):
            hsl = slice(h * 512, (h + 1) * 512)
            e1o_ps = ph1ps.tile([1, 512], f32, tag="e1o", bufs=1)
            for fc in range(2):
                nc.tensor.matmul(
                    e1o_ps,
                    trans_w[:, fc, D : D + 1],
                    xTo_sb[:, fc, hsl],
                    start=(fc == 0),
                    stop=(fc == 1),
                )
            nc.scalar.activation(invc_row[:, hsl], e1o_ps, Exp, scale=-(1.0 - ALPHA))
        for h in range(2):
            hsl = slice(h * 512, (h + 1) * 512)
            ib_ps = ph1ps.tile([128, 512], f32, tag="ib", bufs=1)
            nc.tensor.matmul(ib_ps, ones1, invc_row[:, hsl], start=True, stop=True)
            nc.vector.tensor_copy(invc_rep[:, hsl], ib_ps)

    # ---------------- interleaved pipeline: he-groups (h/e/d/b2/haug/haugD
    # for 4 j-chunks) emitted LAG groups ahead of the phase-2 chunks that
    # consume them, so phase 1 and phase 2 overlap on every engine queue.
    heps = ctx.enter_context(tc.tile_pool(name="heps", bufs=2, space="PSUM"))
    ps_o = ctx.enter_context(tc.tile_pool(name="ps_o", bufs=1, space="PSUM"))
    po1 = ps_o.tile([D + 1, R], f32, tag="po1")
    po2 = ps_o.tile([D + 1, R], f32, tag="po2")
    Relu = mybir.ActivationFunctionType.Relu
    LAG = 4

    def emit_he_group(g):
        he_ps = heps.tile([128, 4, D + 2], f32, tag="he")
        for k in range(4):
            c = g * 4 + k
            csl = slice(c * 128, (c + 1) * 128)
            for fc in range(2):
                nc.tensor.matmul(
                    he_ps[:, k, :],
                    xT_sb[:, fc, csl],
                    trans_w[:, fc, :],
                    start=(fc == 0),
                    stop=(fc == 1),
                )
        csl4 = slice(g * 4, (g + 1) * 4)
        nc.scalar.activation(
            d_col[:, csl4], he_ps[:, :, D + 1], Exp, scale=1.0 - ALPHA
        )
        nc.scalar.activation(
            b2_col[:, csl4], he_ps[:, :, D + 1], Exp, scale=ALPHA
        )
        nc.vector.tensor_scalar(
            b2_col[:, csl4], b2_col[:, csl4], 1.0 / 16.0, None, MULT
        )
        nc.vector.tensor_scalar(
            neg_d_col[:, csl4], d_col[:, csl4], -1.0, None, MULT
        )
        nc.vector.tensor_tensor(
            d2_col[:, csl4], d_col[:, csl4], b2_col[:, csl4], MULT
        )
        nc.vector.tensor_copy(haug[:, csl4, D], b2_col[:, csl4])
        nc.scalar.copy(haugD[:, csl4, D], d2_col[:, csl4])
        for k in range(4):
            c = g * 4 + k
            if c % 2 == 0:
                nc.vector.tensor_scalar(
                    haug[:, c, 0:D],
                    he_ps[:, k, 0:D],
                    b2_col[:, c : c + 1],
                    None,
                    MULT,
                )
            else:
                nc.scalar.activation(
                    haug[:, c, 0:D],
                    he_ps[:, k, 0:D],
                    mybir.ActivationFunctionType.Copy,
                    scale=b2_col[:, c : c + 1],
                )
            if c % 2 == 0:
                nc.scalar.activation(
                    haugD[:, c, 0:D],
                    he_ps[:, k, 0:D],
                    mybir.ActivationFunctionType.Copy,
                    scale=d2_col[:, c : c + 1],
                )
            else:
                nc.vector.tensor_scalar(
                    haugD[:, c, 0:D],
                    he_ps[:, k, 0:D],
                    d2_col[:, c : c + 1],
                    None,
                    MULT,
                )

    def emit_chunk(jc):
        mt = mask_tiles[jc // 4][:, jc % 4, :]
        t = work.tile([128, R], f16, tag="t")
        q = work.tile([128, R], f16, tag="q")
        nc.scalar.activation(
            t[:, 0:CS], invc_rep[:, 0:CS], Relu, bias=neg_d_col[:, jc : jc + 1]
        )
        nc.vector.tensor_scalar(
            t[:, CS:], invc_rep[:, CS:], d_col[:, jc : jc + 1], None, MAX
        )
        nc.vector.tensor_tensor(q[:, 0:DS], t[:, 0:DS], mt[:, 0:DS], MULT)
        nc.gpsimd.tensor_tensor(q[:, DS:], t[:, DS:], mt[:, DS:], MULT)
        for h in range(2):
            hsl = slice(h * 512, (h + 1) * 512)
            nc.tensor.matmul(
                po2[:, hsl],
                haug[:, jc, :],
                q[:, hsl],
                start=(jc == 0),
                stop=(jc == JC - 1),
            )
        nc.tensor.matmul(
            po1[:, 0:CS],
            haugD[:, jc, :],
            mt[:, 0:CS],
            start=(jc == 0),
            stop=(jc == JC - 1),
        )

    for g in range(LAG):
        emit_he_group(g)
    for g in range(JC // 4):
        if g + LAG < JC // 4:
            emit_he_group(g + LAG)
        for k in range(4):
            emit_chunk(g * 4 + k)

    # ---------------- normalize and store
    dn2 = outp.tile([1, R], f32)
    nc.scalar.copy(dn2[:, 0:CS], po2[D : D + 1, 0:CS])
    dn_row = outp.tile([1, R], f16)
    nc.vector.tensor_tensor(
        dn_row[:, 0:CS], po1[D : D + 1, 0:CS], dn2[:, 0:CS], ADD
    )
    nc.vector.tensor_copy(dn_row[:, CS:], po2[D : D + 1, CS:])
    osum = outp.tile([D, R], f32)
    nc.vector.tensor_copy(osum[:, 0:CS], po1[0:D, 0:CS])
    nc.vector.tensor_tensor(
        osum[:, 0:CS], osum[:, 0:CS], po2[0:D, 0:CS], ADD
    )
    nc.scalar.copy(osum[:, CS:], po2[0:D, CS:])
    rr_sb = outp.tile([D, R], f32)
    with tc.tile_pool(name="ps_r", bufs=2, space="PSUM") as ps_r:
        for h in range(2):
            hsl = slice(h * 512, (h + 1) * 512)
            rr_ps = ps_r.tile([D, 512], f32, tag="rr")
            nc.tensor.matmul(
                rr_ps, ones1[:, 0:D], dn_row[:, hsl], start=True, stop=True
            )
            nc.vector.reciprocal_approx_fast(rr_sb[:, hsl], rr_ps)
    o_t = outp.tile([D, R], f32)
    nc.vector.tensor_tensor(o_t, osum, rr_sb, MULT)
    for k in range(4):
        sl = slice(k * 256, (k + 1) * 256)
        nc.sync.dma_start(out=outT[:, sl], in_=o_t[:, sl])


def build_nc():
    nc = bacc.Bacc("TRN2", num_devices=N_CORES)
    xT = nc.dram_tensor("xT", [F_IN, N], f16, kind="ExternalInput")
    xTo = nc.dram_tensor("xTo", [F_IN, R], f16, kind="ExternalInput")
    mT = nc.dram_tensor("mT", [N, R], f16, kind="ExternalInput")
    trans = nc.dram_tensor("trans", [F_IN, D], f16, kind="ExternalInput")
    a12 = nc.dram_tensor("a12", [D, 2], f16, kind="ExternalInput")
    outT = nc.dram_tensor("outT", [D, R], f32, kind="ExternalOutput")
    with ExitStack() as ctx:
        tc = ctx.enter_context(tile.TileContext(nc))
        build_kernel(
            ctx, tc, xT[:, :], xTo[:, :], mT[:, :], trans[:, :], a12[:, :], outT[:, :]
        )
    nc.compile()
    return nc


LAST_RESULTS = None


def kernel(x, mask, trans, attn, _trace=False):
    x = np.asarray(x)
    mask = np.asarray(mask)
    trans = np.asarray(trans)
    attn = np.asarray(attn)

    xT16 = np.ascontiguousarray(x.T.astype(np.float16))
    trans16 = np.ascontiguousarray(trans.astype(np.float16))
    a12_16 = np.ascontiguousarray(
        np.concatenate([attn[:D], attn[D:]], axis=1).astype(np.float16)
    )
    # mask as f16 1.0/0.0: feeds the PE directly (m*D term) and one
    # tensor_tensor (m*t term)
    mf = np.where(mask != 0, np.float16(1.0), np.float16(0.0))

    nc = build_nc()
    in_maps = []
    for c in range(N_CORES):
        rows = slice(c * R, (c + 1) * R)
        in_maps.append(
            {
                "xT": xT16,
                "xTo": np.ascontiguousarray(xT16[:, rows]),
                "mT": np.ascontiguousarray(mf[rows, :].T),
                "trans": trans16,
                "a12": a12_16,
            }
        )
    res = run_bass_kernel_spmd(nc, in_maps, list(range(N_CORES)), trace=_trace)
    global LAST_RESULTS
    LAST_RESULTS = res
    out = np.concatenate(
        [res.results[c]["outT"].T for c in range(N_CORES)], axis=0
    )
    return np.ascontiguousarray(out, dtype=np.float32)


if __name__ == "__main__":
    nc = build_nc()
    print("built OK")


# revision 48
# speedup vs baseline: 1.3308x; 1.0054x over previous
"""Trainium2 Bass kernel for nn_NodeAttentionPerMetaPath (GAT-style node attention).

Reference computation (N=8192, F_IN=256, d=64):
    h      = x @ trans                      # [N, d]
    e1     = h @ attn[:d];  e2 = h @ attn[d:]
    scores = leaky_relu(e1 + e2.T, 0.2)     # [N, N]
    masked = where(mask==0, -1e15, scores)
    out    = softmax(masked, axis=1) @ h    # [N, d]

Sharding: rows of the output across 8 cores (1024 rows each). Every core
receives the full xT (host-transposed, fp16) and computes h/e locally —
no collectives anywhere.

Algebraic restructuring (exact):
    exp(leaky(v)) = max(exp(v), exp(a*v))       (exp monotone, a<1)
    with v = e1[r] + e2[j], dropping per-row factors (softmax-invariant):
      P[r,j] ∝ mask * B2[j] * max(D[j], invC[r])
    where D = exp((1-a)e2), invC = exp(-(1-a)e1), B2 = exp(a*e2).

Layout tricks:
  * mask uploaded PRE-TRANSPOSED per core ([j, r]) and encoded as uint16
    0xFFFF/0x0000, so masking is a bitwise AND; bitcast to int32 halves
    the DVE element count (tensor_tensor runs 1x on trn2 DVE regardless
    of dtype — AND-on-int32 is the only way to get 2 f16/cycle).
  * per 128-j-chunk, D[j] and B2[j] are per-PARTITION scalars:
      v = max(invc_rep, D[j])      one 4x tensor_scalar
      p = v & mask                 one int32 AND (split DVE/GPSIMD)
      po[65, 1024] += haug[:,jc,:].T @ p    (haug = [B2*h | B2], so row
                                             64 of po is the denominator)
    No PE transposes, no PSUM->SBUF drains, no collectives.
"""

import os
from contextlib import ExitStack

import numpy as np

import concourse.bass as bass
import concourse.bacc as bacc
import concourse.mybir as mybir
import concourse.tile as tile
from concourse.bass_utils import run_bass_kernel_spmd
from concourse.masks import make_identity

f32 = mybir.dt.float32
f16 = mybir.dt.float16
u16 = mybir.dt.uint16
f8 = mybir.dt.float8e4
i32 = mybir.dt.int32

Exp = mybir.ActivationFunctionType.Exp
MULT = mybir.AluOpType.mult
MAX = mybir.AluOpType.max
AND = mybir.AluOpType.bitwise_and
ADD = mybir.AluOpType.add

N_CORES = 8
N = 8192
F_IN = 256
D = 64  # F_OUT
ALPHA = 0.2

R = N // N_CORES  # output rows per core
JC = N // 128  # j-chunks

# hybrid column split at CS=512: columns [0:CS] use the two-pass form
# (t = relu(invC-D) on ScalarE; po1 adds the m*D term via matmul), columns
# [CS:R] the one-pass form (t = max(invC, D) on DVE; po1 not needed).
CS = 512
# q = m*t split per chunk: DVE multiplies r-columns [0:DS], GPSIMD [DS:R]
DS = 788

MASK_BUFS = 5  # in-flight mask DMA tiles of 4 j-chunks each


def build_kernel(ctx: ExitStack, tc: tile.TileContext, xT, xTo, mT, trans, a12, outT):
    nc = tc.nc

    singles = ctx.enter_context(tc.tile_pool(name="singles", bufs=1))
    maskp = ctx.enter_context(tc.tile_pool(name="maskp", bufs=MASK_BUFS))
    work = ctx.enter_context(tc.tile_pool(name="work", bufs=8))
    outp = ctx.enter_context(tc.tile_pool(name="outp", bufs=1))

    # ---- persistent tensors
    xT_sb = singles.tile([128, 2, N], f16)
    xTo_sb = singles.tile([128, 2, R], f16)
    # trans_w columns: [trans(0:64) | w1(64) | w2(65)] with w12 = trans @ a12
    trans_w = singles.tile([128, 2, D + 2], f16)
    a12_sb = singles.tile([D, 2], f16)
    ident = singles.tile([128, 128], f16)
    # haug columns: [B2*h(0:64) | B2(64)]; B2 carries a 1/16 so the f16
    # denominator row stays below 65504
    haug = singles.tile([128, JC, D + 1], f16)
    invc_rep = singles.tile([128, R], f16)
    # haugD = D[j] * haug  (the m*D term's stationary)
    haugD = singles.tile([128, JC, D + 1], f16)
    d_col = singles.tile([128, JC], f32)
    d2_col = singles.tile([128, JC], f32)
    neg_d_col = singles.tile([128, JC], f32)
    b2_col = singles.tile([128, JC], f32)
    ones1 = singles.tile([1, 128], f16)

    # xT first: it gates phase 1 (16 split dma_starts spread across queues)
    for k in range(16):
        sl = slice(k * 512, (k + 1) * 512)
        nc.sync.dma_start(
            out=xT_sb[:, :, sl], in_=xT.rearrange("(c p) j -> p c j", p=128)[:, :, sl]
        )
    nc.sync.dma_start(out=xTo_sb, in_=xTo.rearrange("(c p) r -> p c r", p=128))
    nc.sync.dma_start(
        out=trans_w[:, :, 0:D], in_=trans.rearrange("(c p) d -> p c d", p=128)
    )
    nc.sync.dma_start(out=a12_sb, in_=a12[:, :])

    # mask stream: 16 dma_starts of 4 j-chunks each -- small enough that
    # the consumer never stalls on a whole-tile DMA, big enough to amortize
    # descriptor-gen + semaphore propagation. Paced by tile-slot release.
    mask_tiles = []
    for g in range(JC // 4):
        mt = maskp.tile([128, 4, R], f16, tag="mt")
        nc.sync.dma_start(
            out=mt,
            in_=mT[g * 512 : (g + 1) * 512, :].rearrange("(c p) r -> p c r", p=128),
        )
        mask_tiles.append(mt)

    make_identity(nc, ident)
    nc.vector.memset(ones1, 1.0)

    # ---------------- phase 1 prologue: w12 and invC (own rows)
    with (
        tc.tile_pool(name="ph1", bufs=1) as ph1,
        tc.tile_pool(name="ph1ps", bufs=2, space="PSUM") as ph1ps,
    ):
        # w12 = trans @ a12 via PE: transpose trans chunks, then matmul
        transT = ph1.tile([D, 2, 128], f16)
        for fc in range(2):
            tp = ph1ps.tile([D, 128], f16, tag="tp", bufs=1)
            nc.tensor.transpose(tp, trans_w[:, fc, 0:D], ident)
            nc.vector.tensor_copy(transT[:, fc, :], tp)
        w12_ps = ph1ps.tile([128, 2, 2], f32, tag="w12", bufs=1)
        for fc in range(2):
            nc.tensor.matmul(
                w12_ps[:, fc, :], transT[:, fc, :], a12_sb, start=True, stop=True
            )
        nc.vector.tensor_copy(trans_w[:, :, D : D + 2], w12_ps)

        # invC for own rows (it gates every phase-2 op):
        # e1_own = w1.T @ xTo, exp(-0.8*x), broadcast via K=1 ones matmul
        invc_row = ph1.tile([1, R], f16)
        for h in range(2):
            hsl = slice(h * 512, (h + 1) * 512)
            e1o_ps = ph1ps.tile([1, 512], f32, tag="e1o", bufs=1)
            for fc in range(2):
                nc.tensor.matmul(
                    e1o_ps,
                    trans_w[:, fc, D : D + 1],
                    xTo_sb[:, fc, hsl],
                    start=(fc == 0),
                    stop=(fc == 1),
                )
            nc.scalar.activation(invc_row[:, hsl], e1o_ps, Exp, scale=-(1.0 - ALPHA))
        for h in range(2):
            hsl = slice(h * 512, (h + 1) * 512)
            ib_ps = ph1ps.tile([128, 512], f32, tag="ib", bufs=1)
            nc.tensor.matmul(ib_ps, ones1, invc_row[:, hsl], start=True, stop=True)
            nc.vector.tensor_copy(invc_rep[:, hsl], ib_ps)

    # ---------------- interleaved pipeline: he-groups (h/e/d/b2/haug/haugD
    # for 4 j-chunks) emitted LAG groups ahead of the phase-2 chunks that
    # consume them, so phase 1 and phase 2 overlap on every engine queue.
    heps = ctx.enter_context(tc.tile_pool(name="heps", bufs=2, space="PSUM"))
    ps_o = ctx.enter_context(tc.tile_pool(name="ps_o", bufs=1, space="PSUM"))
    po1 = ps_o.tile([D + 1, CS], f32, tag="po1")
    po2 = ps_o.tile([D + 1, R], f32, tag="po2")
    Relu = mybir.ActivationFunctionType.Relu
    LAG = 4

    def emit_he_group(g):
        he_ps = heps.tile([128, 4, D + 2], f32, tag="he")
        for k in range(4):
            c = g * 4 + k
            csl = slice(c * 128, (c + 1) * 128)
            for fc in range(2):
                nc.tensor.matmul(
                    he_ps[:, k, :],
                    xT_sb[:, fc, csl],
                    trans_w[:, fc, :],
                    start=(fc == 0),
                    stop=(fc == 1),
                )
        csl4 = slice(g * 4, (g + 1) * 4)
        nc.scalar.activation(
            d_col[:, csl4], he_ps[:, :, D + 1], Exp, scale=1.0 - ALPHA
        )
        nc.scalar.activation(
            b2_col[:, csl4], he_ps[:, :, D + 1], Exp, scale=ALPHA
        )
        nc.vector.tensor_scalar(
            b2_col[:, csl4], b2_col[:, csl4], 1.0 / 16.0, None, MULT
        )
        nc.vector.tensor_scalar(
            neg_d_col[:, csl4], d_col[:, csl4], -1.0, None, MULT
        )
        nc.vector.tensor_tensor(
            d2_col[:, csl4], d_col[:, csl4], b2_col[:, csl4], MULT
        )
        nc.vector.tensor_copy(haug[:, csl4, D], b2_col[:, csl4])
        nc.scalar.copy(haugD[:, csl4, D], d2_col[:, csl4])
        for k in range(4):
            c = g * 4 + k
            if c % 2 == 0:
                nc.vector.tensor_scalar(
                    haug[:, c, 0:D],
                    he_ps[:, k, 0:D],
                    b2_col[:, c : c + 1],
                    None,
                    MULT,
                )
            else:
                nc.scalar.activation(
                    haug[:, c, 0:D],
                    he_ps[:, k, 0:D],
                    mybir.ActivationFunctionType.Copy,
                    scale=b2_col[:, c : c + 1],
                )
            if c % 2 == 0:
                nc.scalar.activation(
                    haugD[:, c, 0:D],
                    he_ps[:, k, 0:D],
                    mybir.ActivationFunctionType.Copy,
                    scale=d2_col[:, c : c + 1],
                )
            else:
                nc.vector.tensor_scalar(
                    haugD[:, c, 0:D],
                    he_ps[:, k, 0:D],
                    d2_col[:, c : c + 1],
                    None,
                    MULT,
                )

    def emit_chunk(jc):
        mt = mask_tiles[jc // 4][:, jc % 4, :]
        t = work.tile([128, R], f16, tag="t")
        q = work.tile([128, R], f16, tag="q")
        nc.scalar.activation(
            t[:, 0:CS], invc_rep[:, 0:CS], Relu, bias=neg_d_col[:, jc : jc + 1]
        )
        nc.vector.tensor_scalar(
            t[:, CS:], invc_rep[:, CS:], d_col[:, jc : jc + 1], None, MAX
        )
        nc.vector.tensor_tensor(q[:, 0:DS], t[:, 0:DS], mt[:, 0:DS], MULT)
        nc.gpsimd.tensor_tensor(q[:, DS:], t[:, DS:], mt[:, DS:], MULT)
        for h in range(2):
            hsl = slice(h * 512, (h + 1) * 512)
            nc.tensor.matmul(
                po2[:, hsl],
                haug[:, jc, :],
                q[:, hsl],
                start=(jc == 0),
                stop=(jc == JC - 1),
            )
        nc.tensor.matmul(
            po1[:, 0:CS],
            haugD[:, jc, :],
            mt[:, 0:CS],
            start=(jc == 0),
            stop=(jc == JC - 1),
        )

    for g in range(LAG):
        emit_he_group(g)
    for g in range(JC // 4):
        if g + LAG < JC // 4:
            emit_he_group(g + LAG)
        for k in range(4):
            emit_chunk(g * 4 + k)

    # ---------------- normalize and store
    dn2 = outp.tile([1, R], f32)
    dn_row = outp.tile([1, R], f16)
    osum = outp.tile([D, R], f32)
    for k in range(4):
        sl = slice(k * 256, (k + 1) * 256)
        if k < 2:
            nc.scalar.copy(dn2[:, sl], po2[D : D + 1, sl])
            nc.vector.tensor_tensor(
                dn_row[:, sl], po1[D : D + 1, sl], dn2[:, sl], ADD
            )
            nc.vector.tensor_copy(osum[:, sl], po1[0:D, sl])
            nc.vector.tensor_tensor(osum[:, sl], osum[:, sl], po2[0:D, sl], ADD)
        else:
            nc.vector.tensor_copy(dn_row[:, sl], po2[D : D + 1, sl])
            nc.scalar.copy(osum[:, sl], po2[0:D, sl])
    rr_sb = outp.tile([D, R], f32)
    o_t = outp.tile([D, R], f32)
    with tc.tile_pool(name="ps_r", bufs=2, space="PSUM") as ps_r:
        for k in range(4):
            sl = slice(k * 256, (k + 1) * 256)
            rr_ps = ps_r.tile([D, 256], f32, tag="rr")
            nc.tensor.matmul(
                rr_ps, ones1[:, 0:D], dn_row[:, sl], start=True, stop=True
            )
            nc.vector.reciprocal_approx_fast(rr_sb[:, sl], rr_ps)
            nc.vector.tensor_tensor(o_t[:, sl], osum[:, sl], rr_sb[:, sl], MULT)
            nc.sync.dma_start(out=outT[:, sl], in_=o_t[:, sl])


def build_nc():
    nc = bacc.Bacc("TRN2", num_devices=N_CORES)
    xT = nc.dram_tensor("xT", [F_IN, N], f16, kind="ExternalInput")
    xTo = nc.dram_tensor("xTo", [F_IN, R], f16, kind="ExternalInput")
    mT = nc.dram_tensor("mT", [N, R], f16, kind="ExternalInput")
    trans = nc.dram_tensor("trans", [F_IN, D], f16, kind="ExternalInput")
    a12 = nc.dram_tensor("a12", [D, 2], f16, kind="ExternalInput")
    outT = nc.dram_tensor("outT", [D, R], f32, kind="ExternalOutput")
    with ExitStack() as ctx:
        tc = ctx.enter_context(tile.TileContext(nc))
        build_kernel(
            ctx, tc, xT[:, :], xTo[:, :], mT[:, :], trans[:, :], a12[:, :], outT[:, :]
        )
    nc.compile()
    return nc


LAST_RESULTS = None


def kernel(x, mask, trans, attn, _trace=False):
    x = np.asarray(x)
    mask = np.asarray(mask)
    trans = np.asarray(trans)
    attn = np.asarray(attn)

    xT16 = np.ascontiguousarray(x.T.astype(np.float16))
    trans16 = np.ascontiguousarray(trans.astype(np.float16))
    a12_16 = np.ascontiguousarray(
        np.concatenate([attn[:D], attn[D:]], axis=1).astype(np.float16)
    )
    # mask as f16 1.0/0.0: feeds the PE directly (m*D term) and one
    # tensor_tensor (m*t term)
    mf = np.where(mask != 0, np.float16(1.0), np.float16(0.0))

    nc = build_nc()
    in_maps = []
    for c in range(N_CORES):
        rows = slice(c * R, (c + 1) * R)
        in_maps.append(
            {
                "xT": xT16,
                "xTo": np.ascontiguousarray(xT16[:, rows]),
                "mT": np.ascontiguousarray(mf[rows, :].T),
                "trans": trans16,
                "a12": a12_16,
            }
        )
    res = run_bass_kernel_spmd(nc, in_maps, list(range(N_CORES)), trace=_trace)
    global LAST_RESULTS
    LAST_RESULTS = res
    out = np.concatenate(
        [res.results[c]["outT"].T for c in range(N_CORES)], axis=0
    )
    return np.ascontiguousarray(out, dtype=np.float32)


if __name__ == "__main__":
    nc = build_nc()
    print("built OK")
